# revision 31
# baseline (speedup 1.0000x reference)
"""Axial attention (B,H,W,C)=(8,128,128,256), 8 heads, for 8 trn2 NeuronCores.

Sharding: data-parallel over batch B=8 -> one batch element per core.
Per core, two passes over x[b] (x pre-cast to bf16 on the host):
  phase A: attention along H (one sequence per column w), writes
           oh + bout0 + bout1 to a bf16 HBM scratch in (H,W,C) layout.
  phase B: attention along W (one sequence per row h), adds the scratch row
           and writes the final fp32 output row.

Per-sequence math (t=128 tokens, C=256, 8 heads of e=32), all matmuls bf16:
  S^T via XBAR dma-transpose (SBUF->SBUF, no PE/PSUM involved);
  QT/KT = W^T @ ST batched over 4 sequences; V per sequence with a fused
  ones-column per head so the attention denominator falls out of the AV
  matmul; scores computed transposed, 4 heads batched into one PSUM tile so
  a single [128,512] exp on the scalar engine covers them; no
  max-subtraction (scores are O(1): Wq is pre-scaled by e^-0.5 on the host).
"""

import sys

sys.path.insert(0, "/opt/trn_rl_repo")

import numpy as np
import ml_dtypes

import concourse.bass as bass
import concourse.tile as tile
from concourse import mybir
from concourse.bass_utils import run_bass_kernel_spmd
from concourse.vector_clock import ScopedClock

F32 = mybir.dt.float32
BF16 = mybir.dt.bfloat16
AF = mybir.ActivationFunctionType
OP = mybir.AluOpType

H = 128
W = 128
C = 256
HEADS = 8
E = C // HEADS  # 32
T = 128  # sequence length for both axes
G = 4  # sequences processed per group (batched projections)

# --- workaround: this toolchain's codegen accepts at most ONE sync-wait per
# instruction; redistribute extra waits onto preceding same-engine nops. ---

_MAXW = 1


def _patched_drain_and_barrier(self, tick_clock, wait_clock):
    probe = self.nc.sync.nop(nofuse=True)
    wait_clock.add_sem_waits(probe.ins, ScopedClock({None: tick_clock.global_clock}))
    conds = list(probe.ins.sync_info.on_wait)
    probe.ins.sync_info.on_wait = conds[:_MAXW]
    rest = conds[_MAXW:]
    while rest:
        n2 = self.nc.sync.nop(nofuse=True)
        if n2.ins.sync_info is None:
            n2.ins.sync_info = mybir.SyncInfo(on_wait=[], on_update=[])
        n2.ins.sync_info.on_wait = rest[:_MAXW]
        rest = rest[_MAXW:]
    self.nc.sync.drain()
    self.nc.all_engine_barrier()
    popped = self.nc._tile_sem_poison_stack.pop()
    assert popped is self._sem_poison
    self.nc.clear_and_free_semaphores(list(self.sems.allocated().values()))
    self.nc.all_engine_barrier()


tile.TileContext._drain_and_barrier = _patched_drain_and_barrier


_CTRL_OPS = ("InstNoOp", "InstDrain", "InstEventSemaphore", "InstCompareAndBranch")


def _split_waits(nc, limit=_MAXW, compute_limit=1):
    """Hoist extra sync-waits onto fresh nops directly before their owner.

    CTRL-encoded instructions take at most one sync-wait on this toolchain;
    compute/DMA instructions take a few more.
    """
    n_split = 0
    for fn in nc.m.functions:
        for blk in fn.blocks:
            insts = blk.instructions
            out = []
            for inst in insts:
                si = inst.sync_info
                limit = (
                    _MAXW if type(inst).__name__ in _CTRL_OPS else compute_limit
                )
                if si is not None and len(si.on_wait) > limit:
                    waits = list(si.on_wait)
                    extra, keep = waits[:-limit], waits[-limit:]
                    k = 0
                    while extra:
                        nop = mybir.InstNoOp(
                            name=f"{inst.name}-wsplit{k}",
                            engine=inst.engine,
                            bass_nofuse=True,
                            sync_info=mybir.SyncInfo(
                                on_wait=extra[:limit], on_update=[]
                            ),
                        )
                        nc.register_instruction(nop, overwrite=True)
                        out.append(nop)
                        extra = extra[limit:]
                        k += 1
                        n_split += 1
                    si.on_wait = keep
                out.append(inst)
            blk.instructions = out
    return n_split


def _bcast_rows(handle_ap, rows):
    """AP that broadcasts a 1D dram tensor across `rows` partitions."""
    return bass.AP(
        tensor=handle_ap.tensor,
        offset=handle_ap.offset,
        ap=[[0, rows]] + [list(p) for p in handle_ap.ap],
    )


def _free_bcast(ap, n):
    """Append a step-0 free dim of size n to an AP (within-partition bcast)."""
    return bass.AP(
        tensor=ap.tensor,
        offset=ap.offset,
        ap=[list(p) for p in ap.ap] + [[0, n]],
    )


def _build():
    nc = bass.Bass("TRN2", target_bir_lowering=False, debug=False)

    # host-pre-transposed inputs: [group, c, s*128+t] so S^T tiles DMA directly
    xta = nc.dram_tensor("xta", [W // G, C, G * T], BF16, kind="ExternalInput")
    xtc = nc.dram_tensor("xtc", [H // G, C, G * T], BF16, kind="ExternalInput")
    wqkv0 = nc.dram_tensor("wqkv0", [C, 3 * C], BF16, kind="ExternalInput")
    wout0 = nc.dram_tensor("wout0", [C, C], BF16, kind="ExternalInput")
    wqkv1 = nc.dram_tensor("wqkv1", [C, 3 * C], BF16, kind="ExternalInput")
    wout1 = nc.dram_tensor("wout1", [C, C], BF16, kind="ExternalInput")
    bsum = nc.dram_tensor("bsum", [C], F32, kind="ExternalInput")
    out = nc.dram_tensor("out", [H, W, C], F32, kind="ExternalOutput")
    scratch = nc.dram_tensor("ohs", [H, W, C], BF16)

    xta_ap = xta.ap()
    xtc_ap = xtc.ap()
    out_ap = out.ap()
    sc_ap = scratch.ap()
    KC = C // 128  # 2 contraction chunks

    with tile.TileContext(nc) as tc:
        with (
            tc.tile_pool(name="const", bufs=1) as const,
            tc.tile_pool(name="work", bufs=4) as work,
            tc.tile_pool(name="ps", bufs=6, space="PSUM") as ps,
            tc.tile_pool(name="psproj", bufs=2, space="PSUM") as psproj,
        ):
            # ---- constants ----
            ident = const.tile([128, 128], BF16, tag="ident")
            from concourse.masks import make_identity

            make_identity(nc, ident)
            bsum_sb = const.tile([128, C], F32, tag="bsum")
            nc.gpsimd.dma_start(out=bsum_sb, in_=_bcast_rows(bsum.ap(), 128))

            wqkv_sb = {}
            wout_sb = {}
            for ax, (wqkv_d, wout_d) in enumerate(
                [(wqkv0, wout0), (wqkv1, wout1)]
            ):
                wq3 = wqkv_d.ap().rearrange("(k p) n -> k p n", p=128)
                wo2 = wout_d.ap().rearrange("(k p) n -> k p n", p=128)
                for k in range(KC):
                    t_qkv = const.tile([128, 3 * C], BF16, tag=f"wqkv{ax}{k}")
                    nc.gpsimd.dma_start(out=t_qkv, in_=wq3[k])
                    wqkv_sb[ax, k] = t_qkv
                    t_o = const.tile([128, C], BF16, tag=f"wout{ax}{k}")
                    nc.gpsimd.dma_start(out=t_o, in_=wo2[k])
                    wout_sb[ax, k] = t_o

            # persistent V'-buffers: ones columns written once, V columns
            # overwritten every sequence
            NVP = 8
            vp_bufs = []
            for i in range(NVP):
                vpb = const.tile([128, HEADS * (E + 1)], BF16, tag=f"vp{i}")
                nc.gpsimd.memset(vpb, 1.0)
                vp_bufs.append(vpb)

            def axial_pass(ax, n_groups=W // G):
                """ax=0: sequences along H (fixed w). ax=1: along W (fixed h)."""
                for grp in range(n_groups):
                    j0 = grp * G
                    xt_ap = xta_ap if ax == 0 else xtc_ap

                    if ax == 1:
                        ohrow = work.tile([128, G, C], BF16, tag="ohrow")
                        nc.sync.dma_start(
                            out=ohrow,
                            in_=sc_ap[j0 : j0 + G].rearrange("h w c -> w h c"),
                        )
                        og = work.tile([128, G, C], F32, tag="og")
                    else:
                        og = work.tile([128, G, C], BF16, tag="oa")

                    # ---- S^T loads directly (host pre-transposed) ----
                    stb = []
                    for k in range(KC):
                        stb_k = work.tile([128, G * T], BF16, tag=f"stb{k}")
                        nc.sync.dma_start(
                            out=stb_k, in_=xt_ap[grp, k * 128 : (k + 1) * 128, :]
                        )
                        stb.append(stb_k)

                    # ---- QT / KT batched over the group ----
                    qtb = []
                    ktb = []
                    for which, dst in ((0, qtb), (1, ktb)):
                        for m in range(KC):
                            pp = psproj.tile([128, G * T], F32, tag="psproj")
                            for k in range(KC):
                                lhs = wqkv_sb[ax, k][
                                    :, which * C + m * 128 : which * C + (m + 1) * 128
                                ]
                                nc.tensor.matmul(
                                    pp,
                                    lhs,
                                    stb[k],
                                    start=(k == 0),
                                    stop=(k == KC - 1),
                                )
                            sb = work.tile(
                                [128, G * T], BF16, tag=f"qk{which}{m}"
                            )
                            nc.vector.tensor_copy(out=sb, in_=pp)
                            dst.append(sb)

                    # ---- stage-major over the G sequences: every stage emits
                    # all 4 sequences' ops back-to-back so the in-order
                    # engine queues pipeline across sequences ----

                    # V (fused ones column per head)
                    vps_l = []
                    for s in range(G):
                        vps = ps.tile([128, C], F32, tag="ps")
                        for k in range(KC):
                            nc.tensor.matmul(
                                vps,
                                stb[k][:, s * T : (s + 1) * T],
                                wqkv_sb[ax, k][:, 2 * C : 3 * C],
                                start=(k == 0),
                                stop=(k == KC - 1),
                            )
                        vps_l.append(vps)
                    vp_l = []
                    for s in range(G):
                        vp = vp_bufs[(grp * G + s) % NVP]
                        vp3 = vp.rearrange("p (h q) -> p h q", q=E + 1)
                        nc.scalar.activation(
                            out=vp3[:, :, 0:E],
                            in_=vps_l[s].rearrange("p (h e) -> p h e", e=E),
                            func=AF.Copy,
                        )
                        vp_l.append(vp)

                    # attention, 4 heads (one chunk) at a time. Scores are
                    # batched per tile-position q ACROSS the 4 sequences: the
                    # 4 matmuls into one PSUM tile share one PE sub-array
                    # (same tile_position) so they serialize naturally —
                    # concurrent row-tiles never touch the same PSUM bank.
                    ops_l = [None] * G
                    for hg in range(2):
                        # interleave score matmuls and exps so at most ~2
                        # score PSUM tiles are live at a time
                        ebq_l = [None] * 4
                        scq_l = [None] * 4
                        for q in range(4):
                            off = q * E
                            scq = ps.tile([128, G * T], F32, tag="ps")
                            for s in range(G):
                                nc.tensor.matmul(
                                    scq[:, s * T : (s + 1) * T],
                                    ktb[hg][off : off + E, s * T : (s + 1) * T],
                                    qtb[hg][off : off + E, s * T : (s + 1) * T],
                                    start=True,
                                    stop=True,
                                    tile_position=(off, 0),
                                )
                            scq_l[q] = scq
                            if q >= 1:
                                qe = q - 1
                                ebq = work.tile([128, G * T], BF16, tag="eb4")
                                nc.scalar.activation(
                                    out=ebq, in_=scq_l[qe], func=AF.Exp
                                )
                                ebq_l[qe] = ebq
                        ebq = work.tile([128, G * T], BF16, tag="eb4")
                        nc.scalar.activation(out=ebq, in_=scq_l[3], func=AF.Exp)
                        ebq_l[3] = ebq
                        for s in range(G):
                            if ops_l[s] is None:
                                ops_t = ps.tile(
                                    [128, HEADS * (E + 1)], F32, tag="ps"
                                )
                                ops_l[s] = ops_t
                            for q in range(4):
                                h = hg * 4 + q
                                nc.tensor.matmul(
                                    ops_l[s][:, h * (E + 1) : (h + 1) * (E + 1)],
                                    ebq_l[q][:, s * T : (s + 1) * T],
                                    vp_l[s][:, h * (E + 1) : (h + 1) * (E + 1)],
                                    start=True,
                                    stop=True,
                                )

                    # normalize (divide by the fused denominator column)
                    recip_l = []
                    for s in range(G):
                        o3 = ops_l[s].rearrange("p (h q) -> p h q", q=E + 1)
                        recip = work.tile([128, HEADS], F32, tag="recip")
                        nc.vector.reciprocal(out=recip, in_=o3[:, :, E])
                        recip_l.append(recip)
                    onorm_l = []
                    for s in range(G):
                        o3 = ops_l[s].rearrange("p (h q) -> p h q", q=E + 1)
                        onorm = work.tile([128, C], BF16, tag="onorm")
                        nc.vector.tensor_tensor(
                            out=onorm.rearrange("p (h e) -> p h e", e=E),
                            in0=o3[:, :, 0:E],
                            in1=_free_bcast(recip_l[s][:], E),
                            op=OP.mult,
                        )
                        onorm_l.append(onorm)

                    # out projection, sequences processed in PAIRS so the
                    # PSUM->SBUF copy, final matmul tile and og-add each
                    # cover two sequences per instruction
                    otb_l = []
                    for sp in range(G // 2):
                        ot_ps = ps.tile([128, 2 * C], BF16, tag="ps")
                        for si in range(2):
                            s = sp * 2 + si
                            for k in range(KC):
                                nc.tensor.transpose(
                                    ot_ps[
                                        :,
                                        si * C + k * 128 : si * C + (k + 1) * 128,
                                    ],
                                    onorm_l[s][:, k * 128 : (k + 1) * 128],
                                    ident,
                                )
                        otb = work.tile([128, 2 * C], BF16, tag="otb")
                        nc.vector.tensor_copy(out=otb, in_=ot_ps)
                        otb_l.append(otb)
                    fps_l = []
                    for sp in range(G // 2):
                        fps = ps.tile([128, 2 * C], F32, tag="ps")
                        for si in range(2):
                            for k in range(KC):
                                nc.tensor.matmul(
                                    fps[:, si * C : (si + 1) * C],
                                    otb_l[sp][
                                        :,
                                        si * C + k * 128 : si * C + (k + 1) * 128,
                                    ],
                                    wout_sb[ax, k],
                                    start=(k == 0),
                                    stop=(k == KC - 1),
                                )
                        fps_l.append(fps)
                    for sp in range(G // 2):
                        fpv = fps_l[sp].rearrange("p (s c) -> p s c", c=C)
                        if ax == 0:
                            bs = bsum_sb[:]
                            in1 = bass.AP(
                                tensor=bs.tensor,
                                offset=bs.offset,
                                ap=[list(bs.ap[0]), [0, 2], list(bs.ap[1])],
                            )
                        else:
                            in1 = ohrow[:, 2 * sp : 2 * sp + 2, :]
                        nc.vector.tensor_tensor(
                            out=og[:, 2 * sp : 2 * sp + 2, :],
                            in0=fpv,
                            in1=in1,
                            op=OP.add,
                        )

                    if ax == 0:
                        nc.sync.dma_start(out=sc_ap[:, j0 : j0 + G, :], in_=og)
                    else:
                        nc.sync.dma_start(
                            out=out_ap[j0 : j0 + G].rearrange("h w c -> w h c"),
                            in_=og,
                        )

            axial_pass(0)
            axial_pass(1)

    _split_waits(nc)
    return nc


_NC = None


def _get_nc():
    global _NC
    if _NC is None:
        _NC = _build()
    return _NC


def make_in_maps(x, Wq0, Wkv0, Wout0, bout0, Wq1, Wkv1, Wout1, bout1):
    bf = ml_dtypes.bfloat16
    scale = float(E) ** -0.5
    wqkv0 = np.concatenate([Wq0 * scale, Wkv0], axis=1).astype(bf)
    wqkv1 = np.concatenate([Wq1 * scale, Wkv1], axis=1).astype(bf)
    xb = np.asarray(x, dtype=bf)
    shared = {
        "wqkv0": wqkv0,
        "wout0": np.asarray(Wout0, dtype=bf),
        "wqkv1": wqkv1,
        "wout1": np.asarray(Wout1, dtype=bf),
        "bsum": np.asarray(bout0 + bout1, dtype=np.float32),
    }
    maps = []
    for b in range(x.shape[0]):
        e = xb[b]  # (H, W, C)
        # xta[g, c, s*T+h] = x[h, 4g+s, c]  (phase A: sequences along H)
        xta_b = np.ascontiguousarray(
            e.transpose(1, 2, 0).reshape(W // G, G, C, H).transpose(0, 2, 1, 3)
        ).reshape(W // G, C, G * T)
        # xtc[g, c, s*T+w] = x[4g+s, w, c]  (phase B: sequences along W)
        xtc_b = np.ascontiguousarray(
            e.reshape(H // G, G, W, C).transpose(0, 3, 1, 2)
        ).reshape(H // G, C, G * T)
        maps.append({"xta": xta_b, "xtc": xtc_b, **shared})
    return maps


def kernel(x, Wq0, Wkv0, Wout0, bout0, Wq1, Wkv1, Wout1, bout1):
    nc = _get_nc()
    in_maps = make_in_maps(
        np.asarray(x),
        np.asarray(Wq0),
        np.asarray(Wkv0),
        np.asarray(Wout0),
        np.asarray(bout0, dtype=np.float32),
        np.asarray(Wq1),
        np.asarray(Wkv1),
        np.asarray(Wout1),
        np.asarray(bout1, dtype=np.float32),
    )
    res = run_bass_kernel_spmd(nc, in_maps, core_ids=list(range(8)))
    return np.stack([r["out"] for r in res.results]).astype(np.float32)


# revision 52
# speedup vs baseline: 11438.0178x; 11438.0178x over previous
"""Axial attention (B,H,W,C)=(8,128,128,256), 8 heads, for 8 trn2 NeuronCores.

Sharding: data-parallel over batch B=8 -> one batch element per core.
Per core, two passes over x[b] (x pre-cast to bf16 on the host):
  phase A: attention along H (one sequence per column w), writes
           oh + bout0 + bout1 to a bf16 HBM scratch in (H,W,C) layout.
  phase B: attention along W (one sequence per row h), adds the scratch row
           and writes the final fp32 output row.

Per-sequence math (t=128 tokens, C=256, 8 heads of e=32), all matmuls bf16
with fp32 PSUM accumulate:
  S^T tiles DMA straight from HBM (the host pre-transposes x into
  [group, c, seq*128+t] layout, so no on-chip input transpose is needed);
  QT/KT = W^T @ S^T batched over 4 sequences per group; V per sequence with
  a fused ones-column per head so the attention denominator falls out of the
  AV matmul; scores computed transposed (keys on partitions) with K=32
  row-tiled matmuls batched per tile-position across the 4 sequences (same
  PE sub-array => naturally serialized; concurrent row-tiles never share a
  PSUM bank, which crashes the exec unit), giving one [128,512] exp on the
  scalar engine per tile-position; no max-subtraction (scores are O(1): Wq
  is pre-scaled by e^-0.5 on the host). AV accumulates per
  (sequence-pair, head-group) PSUM tiles that are normalized and freed
  immediately, keeping enough PSUM banks open for the next group's
  projections to overlap. Stage-major emission (all 4 sequences per stage)
  keeps the in-order engine queues pipelined.

Toolchain notes: this neuronxcc accepts at most ONE sync-wait per
instruction, so Tile's multi-wait sync is legalized post-scheduling by
hoisting extra waits onto same-engine nops (_split_waits + patched
TileContext drain).
"""

import sys

sys.path.insert(0, "/opt/trn_rl_repo")

import numpy as np
import ml_dtypes

import concourse.bass as bass
import concourse.tile as tile
from concourse import mybir
from concourse.bass_utils import run_bass_kernel_spmd
from concourse.vector_clock import ScopedClock

F32 = mybir.dt.float32
BF16 = mybir.dt.bfloat16
AF = mybir.ActivationFunctionType
OP = mybir.AluOpType

H = 128
W = 128
C = 256
HEADS = 8
E = C // HEADS  # 32
T = 128  # sequence length for both axes
G = 4  # sequences processed per group (batched projections)

# --- workaround: this toolchain's codegen accepts at most ONE sync-wait per
# instruction; redistribute extra waits onto preceding same-engine nops. ---

_MAXW = 1


def _patched_drain_and_barrier(self, tick_clock, wait_clock):
    probe = self.nc.sync.nop(nofuse=True)
    wait_clock.add_sem_waits(probe.ins, ScopedClock({None: tick_clock.global_clock}))
    conds = list(probe.ins.sync_info.on_wait)
    probe.ins.sync_info.on_wait = conds[:_MAXW]
    rest = conds[_MAXW:]
    while rest:
        n2 = self.nc.sync.nop(nofuse=True)
        if n2.ins.sync_info is None:
            n2.ins.sync_info = mybir.SyncInfo(on_wait=[], on_update=[])
        n2.ins.sync_info.on_wait = rest[:_MAXW]
        rest = rest[_MAXW:]
    self.nc.sync.drain()
    self.nc.all_engine_barrier()
    popped = self.nc._tile_sem_poison_stack.pop()
    assert popped is self._sem_poison
    self.nc.clear_and_free_semaphores(list(self.sems.allocated().values()))
    self.nc.all_engine_barrier()


tile.TileContext._drain_and_barrier = _patched_drain_and_barrier


_CTRL_OPS = ("InstNoOp", "InstDrain", "InstEventSemaphore", "InstCompareAndBranch")


def _split_waits(nc, limit=_MAXW, compute_limit=1):
    """Hoist extra sync-waits onto fresh nops directly before their owner.

    CTRL-encoded instructions take at most one sync-wait on this toolchain;
    compute/DMA instructions take a few more.
    """
    n_split = 0
    for fn in nc.m.functions:
        for blk in fn.blocks:
            insts = blk.instructions
            out = []
            for inst in insts:
                si = inst.sync_info
                limit = (
                    _MAXW if type(inst).__name__ in _CTRL_OPS else compute_limit
                )
                if si is not None and len(si.on_wait) > limit:
                    waits = list(si.on_wait)
                    extra, keep = waits[:-limit], waits[-limit:]
                    k = 0
                    while extra:
                        nop = mybir.InstNoOp(
                            name=f"{inst.name}-wsplit{k}",
                            engine=inst.engine,
                            bass_nofuse=True,
                            sync_info=mybir.SyncInfo(
                                on_wait=extra[:limit], on_update=[]
                            ),
                        )
                        nc.register_instruction(nop, overwrite=True)
                        out.append(nop)
                        extra = extra[limit:]
                        k += 1
                        n_split += 1
                    si.on_wait = keep
                out.append(inst)
            blk.instructions = out
    return n_split


def _bcast_rows(handle_ap, rows):
    """AP that broadcasts a 1D dram tensor across `rows` partitions."""
    return bass.AP(
        tensor=handle_ap.tensor,
        offset=handle_ap.offset,
        ap=[[0, rows]] + [list(p) for p in handle_ap.ap],
    )


def _free_bcast(ap, n):
    """Append a step-0 free dim of size n to an AP (within-partition bcast)."""
    return bass.AP(
        tensor=ap.tensor,
        offset=ap.offset,
        ap=[list(p) for p in ap.ap] + [[0, n]],
    )


def _build():
    nc = bass.Bass("TRN2", target_bir_lowering=False, debug=False)

    # host-pre-transposed inputs: [group, c, s*128+t] so S^T tiles DMA directly
    xta = nc.dram_tensor("xta", [W // G, C, G * T], BF16, kind="ExternalInput")
    xtc = nc.dram_tensor("xtc", [H // G, C, G * T], BF16, kind="ExternalInput")
    wqkv0 = nc.dram_tensor("wqkv0", [C, 3 * C], BF16, kind="ExternalInput")
    wout0 = nc.dram_tensor("wout0", [C, C], BF16, kind="ExternalInput")
    wqkv1 = nc.dram_tensor("wqkv1", [C, 3 * C], BF16, kind="ExternalInput")
    wout1 = nc.dram_tensor("wout1", [C, C], BF16, kind="ExternalInput")
    bsum = nc.dram_tensor("bsum", [C], F32, kind="ExternalInput")
    out = nc.dram_tensor("out", [H, W, C], F32, kind="ExternalOutput")
    scratch = nc.dram_tensor("ohs", [H, W, C], BF16)

    xta_ap = xta.ap()
    xtc_ap = xtc.ap()
    out_ap = out.ap()
    sc_ap = scratch.ap()
    KC = C // 128  # 2 contraction chunks

    with tile.TileContext(nc) as tc:
        with (
            tc.tile_pool(name="const", bufs=1) as const,
            tc.tile_pool(name="work", bufs=4) as work,
            tc.tile_pool(name="ps", bufs=5, space="PSUM") as ps,
            tc.tile_pool(name="psproj", bufs=3, space="PSUM") as psproj,
            tc.tile_pool(name="ebp", bufs=8) as ebp,
            tc.tile_pool(name="onp", bufs=8) as onp,
        ):
            # ---- constants ----
            ident = const.tile([128, 128], BF16, tag="ident")
            from concourse.masks import make_identity

            make_identity(nc, ident)
            bsum_sb = const.tile([128, C], F32, tag="bsum")
            nc.gpsimd.dma_start(out=bsum_sb, in_=_bcast_rows(bsum.ap(), 128))

            wqkv_sb = {}
            wout_sb = {}
            for ax, (wqkv_d, wout_d) in enumerate(
                [(wqkv0, wout0), (wqkv1, wout1)]
            ):
                wq3 = wqkv_d.ap().rearrange("(k p) n -> k p n", p=128)
                wo2 = wout_d.ap().rearrange("(k p) n -> k p n", p=128)
                for k in range(KC):
                    t_qkv = const.tile([128, 3 * C], BF16, tag=f"wqkv{ax}{k}")
                    nc.gpsimd.dma_start(out=t_qkv, in_=wq3[k])
                    wqkv_sb[ax, k] = t_qkv
                    t_o = const.tile([128, C], BF16, tag=f"wout{ax}{k}")
                    nc.gpsimd.dma_start(out=t_o, in_=wo2[k])
                    wout_sb[ax, k] = t_o

            # persistent V'-buffers: ones columns written once, V columns
            # overwritten every sequence
            NVP = 8
            vp_bufs = []
            for i in range(NVP):
                vpb = const.tile([128, HEADS * (E + 1)], BF16, tag=f"vp{i}")
                nc.gpsimd.memset(vpb, 1.0)
                vp_bufs.append(vpb)

            def axial_pass(ax, n_groups=W // G):
                """ax=0: sequences along H (fixed w). ax=1: along W (fixed h)."""
                for grp in range(n_groups):
                    j0 = grp * G
                    xt_ap = xta_ap if ax == 0 else xtc_ap

                    if ax == 1:
                        ohrow = work.tile([128, G, C], BF16, tag="ohrow")
                        nc.sync.dma_start(
                            out=ohrow,
                            in_=sc_ap[j0 : j0 + G].rearrange("h w c -> w h c"),
                        )
                        og = work.tile([128, G, C], F32, tag="og")
                    else:
                        og = work.tile([128, G, C], BF16, tag="oa")

                    # ---- S^T loads directly (host pre-transposed) ----
                    stb = []
                    for k in range(KC):
                        stb_k = work.tile([128, G * T], BF16, tag=f"stb{k}")
                        nc.sync.dma_start(
                            out=stb_k, in_=xt_ap[grp, k * 128 : (k + 1) * 128, :]
                        )
                        stb.append(stb_k)

                    # ---- QT / KT batched over the group ----
                    qtb = []
                    ktb = []
                    for which, dst in ((0, qtb), (1, ktb)):
                        for m in range(KC):
                            pp = psproj.tile([128, G * T], F32, tag="psproj")
                            for k in range(KC):
                                lhs = wqkv_sb[ax, k][
                                    :, which * C + m * 128 : which * C + (m + 1) * 128
                                ]
                                nc.tensor.matmul(
                                    pp,
                                    lhs,
                                    stb[k],
                                    start=(k == 0),
                                    stop=(k == KC - 1),
                                )
                            sb = work.tile(
                                [128, G * T], BF16, tag=f"qk{which}{m}"
                            )
                            nc.vector.tensor_copy(out=sb, in_=pp)
                            dst.append(sb)

                    # ---- stage-major over the G sequences: every stage emits
                    # all 4 sequences' ops back-to-back so the in-order
                    # engine queues pipeline across sequences ----

                    # V (fused ones column per head)
                    vps_l = []
                    for s in range(G):
                        vps = psproj.tile([128, C], F32, tag="psproj")
                        for k in range(KC):
                            nc.tensor.matmul(
                                vps,
                                stb[k][:, s * T : (s + 1) * T],
                                wqkv_sb[ax, k][:, 2 * C : 3 * C],
                                start=(k == 0),
                                stop=(k == KC - 1),
                            )
                        vps_l.append(vps)
                    vp_l = []
                    for s in range(G):
                        vp = vp_bufs[(grp * G + s) % NVP]
                        vp3 = vp.rearrange("p (h q) -> p h q", q=E + 1)
                        nc.scalar.activation(
                            out=vp3[:, :, 0:E],
                            in_=vps_l[s].rearrange("p (h e) -> p h e", e=E),
                            func=AF.Copy,
                        )
                        vp_l.append(vp)

                    # attention, 4 heads (one chunk) at a time. Scores are
                    # batched per tile-position q ACROSS the 4 sequences: the
                    # 4 matmuls into one PSUM tile share one PE sub-array
                    # (same tile_position) so they serialize naturally —
                    # concurrent row-tiles never touch the same PSUM bank.
                    # onorm pair tiles [128, (si, c')] written per head-group
                    onorm_l = []
                    for sp in range(G // 2):
                        onorm = onp.tile([128, 2 * C], BF16, tag="onorm")
                        onorm_l.append(onorm)

                    HW4 = 4 * (E + 1)  # 132: one head-group's slice per seq
                    for hg in range(2):
                        # interleave score matmuls and exps so at most ~2
                        # score PSUM tiles are live at a time
                        ebq_l = [None] * 4
                        scq_l = [None] * 4
                        for q in range(4):
                            off = q * E
                            scq = ps.tile([128, G * T], F32, tag="ps")
                            for s in range(G):
                                nc.tensor.matmul(
                                    scq[:, s * T : (s + 1) * T],
                                    ktb[hg][off : off + E, s * T : (s + 1) * T],
                                    qtb[hg][off : off + E, s * T : (s + 1) * T],
                                    start=True,
                                    stop=True,
                                    tile_position=(off, 0),
                                )
                            scq_l[q] = scq
                            if q >= 1:
                                qe = q - 1
                                ebq = ebp.tile([128, G * T], BF16, tag="eb4")
                                nc.scalar.activation(
                                    out=ebq, in_=scq_l[qe], func=AF.Exp
                                )
                                ebq_l[qe] = ebq
                        ebq = ebp.tile([128, G * T], BF16, tag="eb4")
                        nc.scalar.activation(out=ebq, in_=scq_l[3], func=AF.Exp)
                        ebq_l[3] = ebq
                        # AV accumulators per (seq-pair, head-group): one
                        # bank each, freed right after this hg's normalize
                        for sp in range(G // 2):
                            opp = ps.tile([128, 2 * HW4], F32, tag="ps")
                            for si in range(2):
                                s = sp * 2 + si
                                for q in range(4):
                                    nc.tensor.matmul(
                                        opp[
                                            :,
                                            si * HW4
                                            + q * (E + 1) : si * HW4
                                            + (q + 1) * (E + 1),
                                        ],
                                        ebq_l[q][:, s * T : (s + 1) * T],
                                        vp_l[s][
                                            :,
                                            (hg * 4 + q)
                                            * (E + 1) : (hg * 4 + q + 1)
                                            * (E + 1),
                                        ],
                                        start=True,
                                        stop=True,
                                    )
                            # normalize this head-group's half right away
                            o4 = bass.AP(
                                tensor=opp.tensor,
                                offset=opp.offset,
                                ap=[
                                    list(opp.ap[0]),
                                    [HW4, 2],
                                    [E + 1, 4],
                                    [1, E + 1],
                                ],
                            )
                            recip = work.tile([128, 2, 4], F32, tag="recip")
                            nc.vector.reciprocal(out=recip, in_=o4[:, :, :, E])
                            ro = recip[:]
                            rb = bass.AP(
                                tensor=ro.tensor,
                                offset=ro.offset,
                                ap=[list(p) for p in ro.ap] + [[0, E]],
                            )
                            onm = onorm_l[sp][:]
                            out_ap_n = bass.AP(
                                tensor=onm.tensor,
                                offset=onm.offset + hg * 128,
                                ap=[list(onm.ap[0]), [C, 2], [E, 4], [1, E]],
                            )
                            nc.vector.tensor_tensor(
                                out=out_ap_n,
                                in0=o4[:, :, :, 0:E],
                                in1=rb,
                                op=OP.mult,
                            )

                    # out projection, sequences processed in PAIRS so the
                    # PSUM->SBUF copy, final matmul tile and og-add each
                    # cover two sequences per instruction
                    otb_l = []
                    for sp in range(G // 2):
                        ot_ps = ps.tile([128, 2 * C], BF16, tag="ps")
                        for si in range(2):
                            s = sp * 2 + si
                            for k in range(KC):
                                nc.tensor.transpose(
                                    ot_ps[
                                        :,
                                        si * C + k * 128 : si * C + (k + 1) * 128,
                                    ],
                                    onorm_l[sp][
                                        :,
                                        si * C + k * 128 : si * C + (k + 1) * 128,
                                    ],
                                    ident,
                                )
                        otb = work.tile([128, 2 * C], BF16, tag="otb")
                        nc.vector.tensor_copy(out=otb, in_=ot_ps)
                        otb_l.append(otb)
                    fps_l = []
                    for sp in range(G // 2):
                        fps = ps.tile([128, 2 * C], F32, tag="ps")
                        for si in range(2):
                            for k in range(KC):
                                nc.tensor.matmul(
                                    fps[:, si * C : (si + 1) * C],
                                    otb_l[sp][
                                        :,
                                        si * C + k * 128 : si * C + (k + 1) * 128,
                                    ],
                                    wout_sb[ax, k],
                                    start=(k == 0),
                                    stop=(k == KC - 1),
                                )
                        fps_l.append(fps)
                    for sp in range(G // 2):
                        fpv = fps_l[sp].rearrange("p (s c) -> p s c", c=C)
                        if ax == 0:
                            bs = bsum_sb[:]
                            in1 = bass.AP(
                                tensor=bs.tensor,
                                offset=bs.offset,
                                ap=[list(bs.ap[0]), [0, 2], list(bs.ap[1])],
                            )
                        else:
                            in1 = ohrow[:, 2 * sp : 2 * sp + 2, :]
                        nc.vector.tensor_tensor(
                            out=og[:, 2 * sp : 2 * sp + 2, :],
                            in0=fpv,
                            in1=in1,
                            op=OP.add,
                        )

                    if ax == 0:
                        nc.sync.dma_start(out=sc_ap[:, j0 : j0 + G, :], in_=og)
                    else:
                        nc.sync.dma_start(
                            out=out_ap[j0 : j0 + G].rearrange("h w c -> w h c"),
                            in_=og,
                        )

            axial_pass(0)
            axial_pass(1)

    _split_waits(nc)
    return nc


_NC = None


def _get_nc():
    global _NC
    if _NC is None:
        _NC = _build()
    return _NC


def make_in_maps(x, Wq0, Wkv0, Wout0, bout0, Wq1, Wkv1, Wout1, bout1):
    bf = ml_dtypes.bfloat16
    scale = float(E) ** -0.5
    wqkv0 = np.concatenate([Wq0 * scale, Wkv0], axis=1).astype(bf)
    wqkv1 = np.concatenate([Wq1 * scale, Wkv1], axis=1).astype(bf)
    xb = np.asarray(x, dtype=bf)
    shared = {
        "wqkv0": wqkv0,
        "wout0": np.asarray(Wout0, dtype=bf),
        "wqkv1": wqkv1,
        "wout1": np.asarray(Wout1, dtype=bf),
        "bsum": np.asarray(bout0 + bout1, dtype=np.float32),
    }
    maps = []
    for b in range(x.shape[0]):
        e = xb[b]  # (H, W, C)
        # xta[g, c, s*T+h] = x[h, 4g+s, c]  (phase A: sequences along H)
        xta_b = np.ascontiguousarray(
            e.transpose(1, 2, 0).reshape(W // G, G, C, H).transpose(0, 2, 1, 3)
        ).reshape(W // G, C, G * T)
        # xtc[g, c, s*T+w] = x[4g+s, w, c]  (phase B: sequences along W)
        xtc_b = np.ascontiguousarray(
            e.reshape(H // G, G, W, C).transpose(0, 3, 1, 2)
        ).reshape(H // G, C, G * T)
        maps.append({"xta": xta_b, "xtc": xtc_b, **shared})
    return maps


def kernel(x, Wq0, Wkv0, Wout0, bout0, Wq1, Wkv1, Wout1, bout1):
    nc = _get_nc()
    in_maps = make_in_maps(
        np.asarray(x),
        np.asarray(Wq0),
        np.asarray(Wkv0),
        np.asarray(Wout0),
        np.asarray(bout0, dtype=np.float32),
        np.asarray(Wq1),
        np.asarray(Wkv1),
        np.asarray(Wout1),
        np.asarray(bout1, dtype=np.float32),
    )
    res = run_bass_kernel_spmd(nc, in_maps, core_ids=list(range(8)))
    return np.stack([r["out"] for r in res.results]).astype(np.float32)


# revision 56
# speedup vs baseline: 11443.4888x; 1.0005x over previous
"""Axial attention (B,H,W,C)=(8,128,128,256), 8 heads, for 8 trn2 NeuronCores.

Sharding: data-parallel over batch B=8 -> one batch element per core.
Per core, two passes over x[b] (x pre-cast to bf16 on the host):
  phase A: attention along H (one sequence per column w), writes
           oh + bout0 + bout1 to a bf16 HBM scratch in (H,W,C) layout.
  phase B: attention along W (one sequence per row h), adds the scratch row
           and writes the final fp32 output row.

Per-sequence math (t=128 tokens, C=256, 8 heads of e=32), all matmuls bf16:
  S^T via XBAR dma-transpose (SBUF->SBUF, no PE/PSUM involved);
  QT/KT = W^T @ ST batched over 4 sequences; V per sequence with a fused
  ones-column per head so the attention denominator falls out of the AV
  matmul; scores computed transposed, 4 heads batched into one PSUM tile so
  a single [128,512] exp on the scalar engine covers them; no
  max-subtraction (scores are O(1): Wq is pre-scaled by e^-0.5 on the host).
"""

import sys

sys.path.insert(0, "/opt/trn_rl_repo")

import numpy as np
import ml_dtypes

import concourse.bass as bass
import concourse.tile as tile
from concourse import mybir
from concourse.bass_utils import run_bass_kernel_spmd
from concourse.vector_clock import ScopedClock

F32 = mybir.dt.float32
BF16 = mybir.dt.bfloat16
AF = mybir.ActivationFunctionType
OP = mybir.AluOpType

H = 128
W = 128
C = 256
HEADS = 8
E = C // HEADS  # 32
T = 128  # sequence length for both axes
G = 4  # sequences processed per group (batched projections)

# --- workaround: this toolchain's codegen accepts at most ONE sync-wait per
# instruction; redistribute extra waits onto preceding same-engine nops. ---

_MAXW = 1


def _patched_drain_and_barrier(self, tick_clock, wait_clock):
    probe = self.nc.sync.nop(nofuse=True)
    wait_clock.add_sem_waits(probe.ins, ScopedClock({None: tick_clock.global_clock}))
    conds = list(probe.ins.sync_info.on_wait)
    probe.ins.sync_info.on_wait = conds[:_MAXW]
    rest = conds[_MAXW:]
    while rest:
        n2 = self.nc.sync.nop(nofuse=True)
        if n2.ins.sync_info is None:
            n2.ins.sync_info = mybir.SyncInfo(on_wait=[], on_update=[])
        n2.ins.sync_info.on_wait = rest[:_MAXW]
        rest = rest[_MAXW:]
    self.nc.sync.drain()
    self.nc.all_engine_barrier()
    popped = self.nc._tile_sem_poison_stack.pop()
    assert popped is self._sem_poison
    self.nc.clear_and_free_semaphores(list(self.sems.allocated().values()))
    self.nc.all_engine_barrier()


tile.TileContext._drain_and_barrier = _patched_drain_and_barrier


_CTRL_OPS = ("InstNoOp", "InstDrain", "InstEventSemaphore", "InstCompareAndBranch")


def _split_waits(nc, limit=_MAXW, compute_limit=1):
    """Hoist extra sync-waits onto fresh nops directly before their owner.

    CTRL-encoded instructions take at most one sync-wait on this toolchain;
    compute/DMA instructions take a few more.
    """
    n_split = 0
    for fn in nc.m.functions:
        for blk in fn.blocks:
            insts = blk.instructions
            out = []
            for inst in insts:
                si = inst.sync_info
                limit = (
                    _MAXW if type(inst).__name__ in _CTRL_OPS else compute_limit
                )
                if si is not None and len(si.on_wait) > limit:
                    waits = list(si.on_wait)
                    extra, keep = waits[:-limit], waits[-limit:]
                    k = 0
                    while extra:
                        nop = mybir.InstNoOp(
                            name=f"{inst.name}-wsplit{k}",
                            engine=inst.engine,
                            bass_nofuse=True,
                            sync_info=mybir.SyncInfo(
                                on_wait=extra[:limit], on_update=[]
                            ),
                        )
                        nc.register_instruction(nop, overwrite=True)
                        out.append(nop)
                        extra = extra[limit:]
                        k += 1
                        n_split += 1
                    si.on_wait = keep
                out.append(inst)
            blk.instructions = out
    return n_split


def _bcast_rows(handle_ap, rows):
    """AP that broadcasts a 1D dram tensor across `rows` partitions."""
    return bass.AP(
        tensor=handle_ap.tensor,
        offset=handle_ap.offset,
        ap=[[0, rows]] + [list(p) for p in handle_ap.ap],
    )


def _free_bcast(ap, n):
    """Append a step-0 free dim of size n to an AP (within-partition bcast)."""
    return bass.AP(
        tensor=ap.tensor,
        offset=ap.offset,
        ap=[list(p) for p in ap.ap] + [[0, n]],
    )


def _build():
    nc = bass.Bass("TRN2", target_bir_lowering=False, debug=False)

    # host-pre-transposed inputs: [group, c, s*128+t] so S^T tiles DMA directly
    xta = nc.dram_tensor("xta", [W // G, C, G * T], BF16, kind="ExternalInput")
    xtc = nc.dram_tensor("xtc", [H // G, C, G * T], BF16, kind="ExternalInput")
    wqkv0 = nc.dram_tensor("wqkv0", [C, 3 * C], BF16, kind="ExternalInput")
    wout0 = nc.dram_tensor("wout0", [C, C], BF16, kind="ExternalInput")
    wqkv1 = nc.dram_tensor("wqkv1", [C, 3 * C], BF16, kind="ExternalInput")
    wout1 = nc.dram_tensor("wout1", [C, C], BF16, kind="ExternalInput")
    bsum = nc.dram_tensor("bsum", [C], F32, kind="ExternalInput")
    out = nc.dram_tensor("out", [H, W, C], F32, kind="ExternalOutput")
    scratch = nc.dram_tensor("ohs", [H, W, C], BF16)

    xta_ap = xta.ap()
    xtc_ap = xtc.ap()
    out_ap = out.ap()
    sc_ap = scratch.ap()
    KC = C // 128  # 2 contraction chunks

    with tile.TileContext(nc) as tc:
        with (
            tc.tile_pool(name="const", bufs=1) as const,
            tc.tile_pool(name="work", bufs=4) as work,
            tc.tile_pool(name="ps", bufs=5, space="PSUM") as ps,
            tc.tile_pool(name="psproj", bufs=3, space="PSUM") as psproj,
            tc.tile_pool(name="ebp", bufs=8) as ebp,
            tc.tile_pool(name="stbp", bufs=3) as stbp,
            tc.tile_pool(name="onp", bufs=8) as onp,
        ):
            # ---- constants ----
            ident = const.tile([128, 128], BF16, tag="ident")
            from concourse.masks import make_identity

            make_identity(nc, ident)
            bsum_sb = const.tile([128, C], F32, tag="bsum")
            nc.gpsimd.dma_start(out=bsum_sb, in_=_bcast_rows(bsum.ap(), 128))

            wqkv_sb = {}
            wout_sb = {}
            for ax, (wqkv_d, wout_d) in enumerate(
                [(wqkv0, wout0), (wqkv1, wout1)]
            ):
                wq3 = wqkv_d.ap().rearrange("(k p) n -> k p n", p=128)
                wo2 = wout_d.ap().rearrange("(k p) n -> k p n", p=128)
                for k in range(KC):
                    t_qkv = const.tile([128, 3 * C], BF16, tag=f"wqkv{ax}{k}")
                    nc.gpsimd.dma_start(out=t_qkv, in_=wq3[k])
                    wqkv_sb[ax, k] = t_qkv
                    t_o = const.tile([128, C], BF16, tag=f"wout{ax}{k}")
                    nc.gpsimd.dma_start(out=t_o, in_=wo2[k])
                    wout_sb[ax, k] = t_o

            # persistent V'-buffers: ones columns written once, V columns
            # overwritten every sequence
            NVP = 8
            vp_bufs = []
            for i in range(NVP):
                vpb = const.tile([128, HEADS * (E + 1)], BF16, tag=f"vp{i}")
                nc.gpsimd.memset(vpb, 1.0)
                vp_bufs.append(vpb)

            def axial_pass(ax, n_groups=W // G):
                """ax=0: sequences along H (fixed w). ax=1: along W (fixed h)."""
                for grp in range(n_groups):
                    j0 = grp * G
                    xt_ap = xta_ap if ax == 0 else xtc_ap

                    if ax == 1:
                        ohrow = work.tile([128, G, C], BF16, tag="ohrow")
                        nc.sync.dma_start(
                            out=ohrow,
                            in_=sc_ap[j0 : j0 + G].rearrange("h w c -> w h c"),
                        )
                        og = work.tile([128, G, C], F32, tag="og")
                    else:
                        og = work.tile([128, G, C], BF16, tag="oa")

                    # ---- S^T loads directly (host pre-transposed) ----
                    stb = []
                    for k in range(KC):
                        stb_k = stbp.tile([128, G * T], BF16, tag=f"stb{k}")
                        nc.sync.dma_start(
                            out=stb_k, in_=xt_ap[grp, k * 128 : (k + 1) * 128, :]
                        )
                        stb.append(stb_k)

                    # ---- QT / KT batched over the group ----
                    qtb = []
                    ktb = []
                    for which, dst in ((0, qtb), (1, ktb)):
                        for m in range(KC):
                            pp = psproj.tile([128, G * T], F32, tag="psproj")
                            for k in range(KC):
                                lhs = wqkv_sb[ax, k][
                                    :, which * C + m * 128 : which * C + (m + 1) * 128
                                ]
                                nc.tensor.matmul(
                                    pp,
                                    lhs,
                                    stb[k],
                                    start=(k == 0),
                                    stop=(k == KC - 1),
                                )
                            sb = work.tile(
                                [128, G * T], BF16, tag=f"qk{which}{m}"
                            )
                            nc.vector.tensor_copy(out=sb, in_=pp)
                            dst.append(sb)

                    # ---- stage-major over the G sequences: every stage emits
                    # all 4 sequences' ops back-to-back so the in-order
                    # engine queues pipeline across sequences ----

                    # V (fused ones column per head)
                    vps_l = []
                    for s in range(G):
                        vps = psproj.tile([128, C], F32, tag="psproj")
                        for k in range(KC):
                            nc.tensor.matmul(
                                vps,
                                stb[k][:, s * T : (s + 1) * T],
                                wqkv_sb[ax, k][:, 2 * C : 3 * C],
                                start=(k == 0),
                                stop=(k == KC - 1),
                            )
                        vps_l.append(vps)
                    vp_l = []
                    for s in range(G):
                        vp = vp_bufs[(grp * G + s) % NVP]
                        vp3 = vp.rearrange("p (h q) -> p h q", q=E + 1)
                        nc.scalar.activation(
                            out=vp3[:, :, 0:E],
                            in_=vps_l[s].rearrange("p (h e) -> p h e", e=E),
                            func=AF.Copy,
                        )
                        vp_l.append(vp)

                    # attention, 4 heads (one chunk) at a time. Scores are
                    # batched per tile-position q ACROSS the 4 sequences: the
                    # 4 matmuls into one PSUM tile share one PE sub-array
                    # (same tile_position) so they serialize naturally —
                    # concurrent row-tiles never touch the same PSUM bank.
                    # onorm pair tiles [128, (si, c')] written per head-group
                    onorm_l = []
                    for sp in range(G // 2):
                        onorm = onp.tile([128, 2 * C], BF16, tag="onorm")
                        onorm_l.append(onorm)

                    HW4 = 4 * (E + 1)  # 132: one head-group's slice per seq
                    for hg in range(2):
                        # interleave score matmuls and exps so at most ~2
                        # score PSUM tiles are live at a time
                        ebq_l = [None] * 4
                        scq_l = [None] * 4
                        for q in range(4):
                            off = q * E
                            scq = ps.tile([128, G * T], F32, tag="ps")
                            for s in range(G):
                                nc.tensor.matmul(
                                    scq[:, s * T : (s + 1) * T],
                                    ktb[hg][off : off + E, s * T : (s + 1) * T],
                                    qtb[hg][off : off + E, s * T : (s + 1) * T],
                                    start=True,
                                    stop=True,
                                    tile_position=(off, 0),
                                )
                            scq_l[q] = scq
                            if q >= 1:
                                qe = q - 1
                                ebq = ebp.tile([128, G * T], BF16, tag="eb4")
                                nc.scalar.activation(
                                    out=ebq, in_=scq_l[qe], func=AF.Exp
                                )
                                ebq_l[qe] = ebq
                        ebq = ebp.tile([128, G * T], BF16, tag="eb4")
                        nc.scalar.activation(out=ebq, in_=scq_l[3], func=AF.Exp)
                        ebq_l[3] = ebq
                        # AV accumulators per (seq-pair, head-group): one
                        # bank each, freed right after this hg's normalize
                        for sp in range(G // 2):
                            opp = ps.tile([128, 2 * HW4], F32, tag="ps")
                            for si in range(2):
                                s = sp * 2 + si
                                for q in range(4):
                                    nc.tensor.matmul(
                                        opp[
                                            :,
                                            si * HW4
                                            + q * (E + 1) : si * HW4
                                            + (q + 1) * (E + 1),
                                        ],
                                        ebq_l[q][:, s * T : (s + 1) * T],
                                        vp_l[s][
                                            :,
                                            (hg * 4 + q)
                                            * (E + 1) : (hg * 4 + q + 1)
                                            * (E + 1),
                                        ],
                                        start=True,
                                        stop=True,
                                    )
                            # normalize this head-group's half right away
                            o4 = bass.AP(
                                tensor=opp.tensor,
                                offset=opp.offset,
                                ap=[
                                    list(opp.ap[0]),
                                    [HW4, 2],
                                    [E + 1, 4],
                                    [1, E + 1],
                                ],
                            )
                            recip = work.tile([128, 2, 4], F32, tag="recip")
                            nc.vector.reciprocal(out=recip, in_=o4[:, :, :, E])
                            ro = recip[:]
                            rb = bass.AP(
                                tensor=ro.tensor,
                                offset=ro.offset,
                                ap=[list(p) for p in ro.ap] + [[0, E]],
                            )
                            onm = onorm_l[sp][:]
                            out_ap_n = bass.AP(
                                tensor=onm.tensor,
                                offset=onm.offset + hg * 128,
                                ap=[list(onm.ap[0]), [C, 2], [E, 4], [1, E]],
                            )
                            nc.vector.tensor_tensor(
                                out=out_ap_n,
                                in0=o4[:, :, :, 0:E],
                                in1=rb,
                                op=OP.mult,
                            )

                    # out projection, sequences processed in PAIRS so the
                    # PSUM->SBUF copy, final matmul tile and og-add each
                    # cover two sequences per instruction
                    otb_l = []
                    for sp in range(G // 2):
                        ot_ps = ps.tile([128, 2 * C], BF16, tag="ps")
                        for si in range(2):
                            s = sp * 2 + si
                            for k in range(KC):
                                nc.tensor.transpose(
                                    ot_ps[
                                        :,
                                        si * C + k * 128 : si * C + (k + 1) * 128,
                                    ],
                                    onorm_l[sp][
                                        :,
                                        si * C + k * 128 : si * C + (k + 1) * 128,
                                    ],
                                    ident,
                                )
                        otb = work.tile([128, 2 * C], BF16, tag="otb")
                        nc.vector.tensor_copy(out=otb, in_=ot_ps)
                        otb_l.append(otb)
                    fps_l = []
                    for sp in range(G // 2):
                        fps = ps.tile([128, 2 * C], F32, tag="ps")
                        for si in range(2):
                            for k in range(KC):
                                nc.tensor.matmul(
                                    fps[:, si * C : (si + 1) * C],
                                    otb_l[sp][
                                        :,
                                        si * C + k * 128 : si * C + (k + 1) * 128,
                                    ],
                                    wout_sb[ax, k],
                                    start=(k == 0),
                                    stop=(k == KC - 1),
                                )
                        fps_l.append(fps)
                    for sp in range(G // 2):
                        fpv = fps_l[sp].rearrange("p (s c) -> p s c", c=C)
                        if ax == 0:
                            bs = bsum_sb[:]
                            in1 = bass.AP(
                                tensor=bs.tensor,
                                offset=bs.offset,
                                ap=[list(bs.ap[0]), [0, 2], list(bs.ap[1])],
                            )
                        else:
                            in1 = ohrow[:, 2 * sp : 2 * sp + 2, :]
                        nc.vector.tensor_tensor(
                            out=og[:, 2 * sp : 2 * sp + 2, :],
                            in0=fpv,
                            in1=in1,
                            op=OP.add,
                        )

                    if ax == 0:
                        nc.sync.dma_start(out=sc_ap[:, j0 : j0 + G, :], in_=og)
                    else:
                        nc.sync.dma_start(
                            out=out_ap[j0 : j0 + G].rearrange("h w c -> w h c"),
                            in_=og,
                        )

            axial_pass(0)
            axial_pass(1)

    _split_waits(nc)
    return nc


_NC = None


def _get_nc():
    global _NC
    if _NC is None:
        _NC = _build()
    return _NC


def make_in_maps(x, Wq0, Wkv0, Wout0, bout0, Wq1, Wkv1, Wout1, bout1):
    bf = ml_dtypes.bfloat16
    scale = float(E) ** -0.5
    wqkv0 = np.concatenate([Wq0 * scale, Wkv0], axis=1).astype(bf)
    wqkv1 = np.concatenate([Wq1 * scale, Wkv1], axis=1).astype(bf)
    xb = np.asarray(x, dtype=bf)
    shared = {
        "wqkv0": wqkv0,
        "wout0": np.asarray(Wout0, dtype=bf),
        "wqkv1": wqkv1,
        "wout1": np.asarray(Wout1, dtype=bf),
        "bsum": np.asarray(bout0 + bout1, dtype=np.float32),
    }
    maps = []
    for b in range(x.shape[0]):
        e = xb[b]  # (H, W, C)
        # xta[g, c, s*T+h] = x[h, 4g+s, c]  (phase A: sequences along H)
        xta_b = np.ascontiguousarray(
            e.transpose(1, 2, 0).reshape(W // G, G, C, H).transpose(0, 2, 1, 3)
        ).reshape(W // G, C, G * T)
        # xtc[g, c, s*T+w] = x[4g+s, w, c]  (phase B: sequences along W)
        xtc_b = np.ascontiguousarray(
            e.reshape(H // G, G, W, C).transpose(0, 3, 1, 2)
        ).reshape(H // G, C, G * T)
        maps.append({"xta": xta_b, "xtc": xtc_b, **shared})
    return maps


def kernel(x, Wq0, Wkv0, Wout0, bout0, Wq1, Wkv1, Wout1, bout1):
    nc = _get_nc()
    in_maps = make_in_maps(
        np.asarray(x),
        np.asarray(Wq0),
        np.asarray(Wkv0),
        np.asarray(Wout0),
        np.asarray(bout0, dtype=np.float32),
        np.asarray(Wq1),
        np.asarray(Wkv1),
        np.asarray(Wout1),
        np.asarray(bout1, dtype=np.float32),
    )
    res = run_bass_kernel_spmd(nc, in_maps, core_ids=list(range(8)))
    return np.stack([r["out"] for r in res.results]).astype(np.float32)
